# revision 71
# baseline (speedup 1.0000x reference)
"""Trainium2 Bass kernel for nn_EventWarping (contrast-maximization event
warping loss).

Strategy (data-parallel over batch, one NeuronCore per batch element):
  Bilinear scatter-add of N=262144 warped events into 256x256 images via the
  TensorEngine outer-product histogram: per chunk of 128 events,
    image[y, (x|x*ts)] += ty[e, y]^T @ rhs[e, (x|x*ts)]
  where ty is the 2-tap y-tent and rhs packs the x-tent and its ts-weighted
  copy.  Events are polarity-sorted on the host (scatter-add is permutation
  invariant), so each chunk touches only its polarity's 4 PSUM banks,
  halving the matmul count vs an unsorted kernel.
  Per chunk, one GpSimd local_scatter builds warp-1's y-tent, warp-0's rhs
  and warp-1's den half into the first 1024 columns of a [128,1280] tile
  from precomputed int16 indices and bf16 data (8 taps per event,
  out-of-bounds taps mapped to negative indices which local_scatter drops).
  local_scatter's cost is dominated by zeroing its prefix, so warp-1's num
  half [1024:1280] is instead fully overwritten on DVE as den1 * tsw, and
  warp-0's y-tent is built in parallel as relu(min(1-d, 1+d)) with two DVE
  ops (sharing one per-event pointer scalar) and one ACT relu.  The PE runs
  4 (ldweights+matmul) pairs per chunk into 8 PSUM banks.  24 chunks per
  hardware-loop iteration amortize the ~2us all-engine back-edge barrier
  (loop pool bufs=1: the barrier already serializes iterations).
  Field prep (warp positions, floors, tap indices/weights) runs as four
  quarter-width DVE passes: the pos-segment quarters before the pos loop,
  the neg-segment quarters emitted between the loops so they overlap the
  pos loop's Pool/PE time.
  Epilogue computes sum((num/(den+eps))^2)/mt^2/nonzero_px per warp plus the
  Charbonnier flow-smoothness term on device; host sums the 8 per-core
  losses.
"""

import sys

if "/opt/trn_rl_repo" not in sys.path:
    sys.path.insert(0, "/opt/trn_rl_repo")

from contextlib import ExitStack

import ml_dtypes
import numpy as np

import concourse.bacc as bacc
import concourse.bass as bass
import concourse.mybir as mybir
from concourse.tile import TileContext

F32 = mybir.dt.float32
BF16 = mybir.dt.bfloat16
I16 = mybir.dt.int16
I32 = mybir.dt.int32
AL = mybir.AluOpType
ACTF = mybir.ActivationFunctionType

P = 128
RES = 256
NPIX = RES * RES
EPS = 1e-9
FLOW_TEMP_REG = 1e-3
PAD_POS = -1e4  # pad-event coordinate: far out of bounds, zero contribution

NT = 8    # scatter taps: 2 y-taps (warp1) + 4 x-taps (warp0) + 2 den taps (warp1)
SW = 1280  # scatter tile width: ty1[256] rhs0[512] den1[256] num1[256]
SZ = 1024  # zero+scatter region; num1 [1024:1280] is DVE-built (den1 * tsw)
U = 24    # chunks per hw-loop iteration
CB = 120  # chunks per block (dynamic-offset register range: 4*24*16B < 2KB)


def _emit(tc, ev, iotas, vecb, loss_out, C2, mt):
    """C2 = total chunks (pos segment PC + neg segment PC, PC = C2//2)."""
    nc = tc.nc
    PC = C2 // 2
    stk = ExitStack()

    const_pool = stk.enter_context(tc.tile_pool(name="const", bufs=1))
    iota = const_pool.tile([P, 256], BF16)
    nc.sync.dma_start(iota, iotas[:, 0:256])
    iota_n = const_pool.tile([P, 256], BF16)
    nc.sync.dma_start(iota_n, iotas[:, 256:512])
    ones = const_pool.tile([P, 1], F32)
    nc.gpsimd.memset(ones, 1.0)
    zk = const_pool.tile([1, 640], BF16)
    nc.gpsimd.memset(zk, 0.0)
    vtile = const_pool.tile([1, 32], F32)
    nc.sync.dma_start(vtile, vecb)

    # persistent per-event scatter operands: NT taps per event, interleaved
    # per chunk column: idxw/datw[p, NT*c + k], plus warp0 y-tent ptr scalars
    pers_pool = stk.enter_context(tc.tile_pool(name="pers", bufs=1))
    idxw = pers_pool.tile([P, NT * C2], I16)
    datw = pers_pool.tile([P, NT * C2], BF16)
    wy1m0 = pers_pool.tile([P, C2], F32)  # 1 - wy (warp0)
    tsw1 = pers_pool.tile([P, C2], F32)   # mt - ts (warp1 num weight)
    idx4 = idxw[:, 0 : NT * C2].rearrange("p (c f) -> p c f", f=NT)
    dat4 = datw[:, 0 : NT * C2].rearrange("p (c f) -> p c f", f=NT)

    psum_pool = tc.tile_pool(name="psum", bufs=1, space="PSUM")
    psum = psum_pool.__enter__()
    # IMG[pol][w][h]: den in cols 0:256, num in cols 256:512
    IMG = [
        [
            [psum.tile([P, 512], F32, tag=f"I{p_}{w}{h}", name=f"I{p_}{w}{h}")
             for h in (0, 1)]
            for w in (0, 1)
        ]
        for p_ in (0, 1)
    ]

    zl = zk[0:1, 0:128]
    zr = zk[0:1, 128:640]
    for p_ in (0, 1):
        for w in (0, 1):
            for h in (0, 1):
                nc.tensor.matmul(out=IMG[p_][w][h][:], lhsT=zl, rhs=zr,
                                 start=True, stop=False)

    # bufs=1: the default back-edge all-engine barrier already serializes
    # iterations, so cross-iteration double buffering buys nothing; the 24
    # per-u tile sets provide all the within-iteration pipelining.
    loop_pool_cm = tc.tile_pool(name="loop", bufs=1)
    loop_pool = loop_pool_cm.__enter__()

    # ---- prep: [P, CH]-wide field math on DVE, in four quarter passes.
    # Quarters 0-1 (pos segment) run before the pos loop; quarters 2-3 (neg
    # segment) are emitted after it so their DVE work hides under the pos
    # loop's Pool/PE time.  The raw pool stays open across the pos loop
    # (quarter-width tiles keep the combined SBUF footprint under budget).
    CH = C2 // 4
    raw_cm = tc.tile_pool(name="raw", bufs=1)
    raw_pool = raw_cm.__enter__()
    ts_t = raw_pool.tile([P, CH], F32, tag="rts", name="rts")
    y_t = raw_pool.tile([P, CH], F32, tag="ry", name="ry")
    x_t = raw_pool.tile([P, CH], F32, tag="rx", name="rx")
    fy_t = raw_pool.tile([P, CH], F32, tag="rfy", name="rfy")
    fx_t = raw_pool.tile([P, CH], F32, tag="rfx", name="rfx")
    d0 = raw_pool.tile([P, CH], F32, tag="d0")
    scr = raw_pool.tile([P, CH], F32, tag="scr")
    scr2 = raw_pool.tile([P, CH], F32, tag="scr2")
    scri = raw_pool.tile([P, CH], I32, tag="scri")
    wp = [[raw_pool.tile([P, CH], F32, tag=f"wp{w}{a}", name=f"wp{w}{a}")
           for a in (0, 1)] for w in (0, 1)]
    # aliases for x-prep scratch (fields are dead by then within a pass)
    flr = y_t
    f1 = fy_t
    m0 = fx_t
    m1 = x_t

    def emit_prep(q):
        lo = q * CH
        idh = idx4[:, lo : lo + CH, :]
        dah = dat4[:, lo : lo + CH, :]
        for f, t in enumerate((ts_t, y_t, x_t, fy_t, fx_t)):
            src = ev[f : f + 1, :].rearrange("o (p c) -> (o p) c", p=P)
            nc.sync.dma_start(t, src[:, lo : lo + CH])

        nc.vector.tensor_scalar(d0, ts_t, -1.0, float(mt), AL.mult, AL.add)
        for w, tw in ((0, d0), (1, ts_t)):
            op = AL.add if w == 0 else AL.subtract
            nc.vector.tensor_tensor(out=scr, in0=tw, in1=fy_t, op=AL.mult)
            nc.vector.tensor_tensor(out=wp[w][0], in0=y_t, in1=scr, op=op)
            nc.vector.tensor_tensor(out=scr, in0=tw, in1=fx_t, op=AL.mult)
            nc.vector.tensor_tensor(out=wp[w][1], in0=x_t, in1=scr, op=op)

        # warp0 y-tent ptr scalar; warp1 num weight
        nc.vector.tensor_scalar(wy1m0[:, lo : lo + CH], wp[0][0], -1.0, 1.0,
                                AL.mult, AL.add)
        nc.vector.tensor_copy(out=tsw1[:, lo : lo + CH], in_=d0)

        # tap layout per event (offsets into the [P,SW] scatter tile):
        #  0: y0   w1 (off 0)      1: y0+1 w1
        #  2: x0   w0 den (256)    3: x0+1 w0 den
        #  4: x0   w0 num (512)    5: x0+1 w0 num
        #  6: x0   w1 den (768)    7: x0+1 w1 den
        # (w1 num [1024:1280] = den1 * tsw1, built on DVE in the loop)
        for w in (0, 1):
            tw = ts_t if w == 0 else d0
            for a in (0, 1):
                if a == 0 and w == 0:
                    continue  # warp0 y-tent built on DVE/ACT in the loop
                xc = wp[w][a]
                nc.vector.tensor_scalar(xc, xc, 300.0, -4.0, AL.min, AL.max)
                nc.vector.tensor_copy(out=scri, in_=xc)
                nc.vector.tensor_copy(out=flr, in_=scri)
                nc.vector.tensor_tensor(out=scr, in0=flr, in1=xc, op=AL.is_gt)
                nc.vector.tensor_tensor(out=flr, in0=flr, in1=scr,
                                        op=AL.subtract)
                nc.vector.tensor_tensor(out=f1, in0=xc, in1=flr,
                                        op=AL.subtract)
                # masks: m_k = (flr+k <= 255) & (flr+k >= 0 when offset > 0)
                nc.vector.tensor_scalar(m0, flr, 255.5, None, AL.is_le)
                nc.vector.tensor_scalar(m1, flr, 254.5, None, AL.is_le)
                if a == 0:
                    offs = [0]
                elif w == 0:
                    offs = [256, 512]
                else:
                    offs = [768]
                need_lower = offs != [0]
                if need_lower:
                    nc.vector.tensor_scalar(scr2, flr, -0.5, None, AL.is_ge)
                    nc.vector.tensor_tensor(out=m0, in0=m0, in1=scr2,
                                            op=AL.mult)
                    nc.vector.tensor_scalar(scr2, flr, -1.5, None, AL.is_ge)
                    nc.vector.tensor_tensor(out=m1, in0=m1, in1=scr2,
                                            op=AL.mult)
                base_k = 0 if a == 0 else (2 + 4 * w)
                for j, off in enumerate(offs):
                    # i0 = (flr+off+1)*m0 - 1 ; i1 = (flr+off+2)*m1 - 1
                    nc.vector.tensor_scalar(scr, flr, float(off + 1), None,
                                            AL.add)
                    nc.vector.tensor_tensor(out=scr, in0=scr, in1=m0,
                                            op=AL.mult)
                    nc.vector.tensor_scalar(scr, scr, -1.0, None, AL.add)
                    nc.vector.tensor_copy(out=idh[:, :, base_k + 2 * j],
                                          in_=scr)
                    nc.vector.tensor_scalar(scr, flr, float(off + 2), None,
                                            AL.add)
                    nc.vector.tensor_tensor(out=scr, in0=scr, in1=m1,
                                            op=AL.mult)
                    nc.vector.tensor_scalar(scr, scr, -1.0, None, AL.add)
                    nc.vector.tensor_copy(out=idh[:, :, base_k + 2 * j + 1],
                                          in_=scr)
                # data: f0 = 1-f1 at tap0, f1 at tap1 (den); *tw for num taps
                nc.vector.tensor_scalar(scr, f1, -1.0, 1.0, AL.mult, AL.add)
                nc.vector.tensor_copy(out=dah[:, :, base_k], in_=scr)
                nc.vector.tensor_copy(out=dah[:, :, base_k + 1], in_=f1)
                if a == 1 and w == 0:
                    nc.vector.tensor_tensor(out=scr, in0=scr, in1=tw,
                                            op=AL.mult)
                    nc.vector.tensor_copy(out=dah[:, :, base_k + 2], in_=scr)
                    nc.vector.tensor_tensor(out=scr, in0=f1, in1=tw,
                                            op=AL.mult)
                    nc.vector.tensor_copy(out=dah[:, :, base_k + 3], in_=scr)

    emit_prep(0)
    emit_prep(1)

    def chunk_body(pol, base, span, i, u):
        iv = idx4[:, base + u : base + span : U, :][:, bass.ds(i, 1), :]
        dv = dat4[:, base + u : base + span : U, :][:, bass.ds(i, 1), :]
        scat = loop_pool.tile([P, SW], BF16, tag=f"s{u}", name=f"s{u}")
        nc.gpsimd.local_scatter(scat[:, 0:SZ], dv, iv, channels=P,
                                num_elems=SZ, num_idxs=NT)
        # warp1 num half: den1 * tsw1 (fully overwrites [1024:1280])
        t1c = tsw1[:, base + u : base + span : U][:, bass.ds(i, 1)]
        nc.vector.tensor_scalar(scat[:, 1024:1280], scat[:, 768:1024], t1c,
                                None, AL.mult)
        # warp0 y-tent: ut = 1+d, tp = min(1-d, 1+d), ty0 = relu(tp)
        # (iota_n holds 2-iota so both ops share the single wy1m scalar)
        w1c = wy1m0[:, base + u : base + span : U][:, bass.ds(i, 1)]
        ut = loop_pool.tile([P, 256], BF16, tag=f"u{u}", name=f"u{u}")
        nc.vector.tensor_scalar(ut, iota, w1c, None, AL.add)
        nc.vector.scalar_tensor_tensor(ut, iota_n, w1c, ut, AL.subtract, AL.min)
        ty0 = loop_pool.tile([P, 256], BF16, tag=f"t{u}", name=f"t{u}")
        nc.scalar.activation(ty0, ut, ACTF.Relu)
        for w in (0, 1):
            rhs = scat[:, 256 + 512 * w : 768 + 512 * w]
            for h in (0, 1):
                lhsT = (ty0[:, 128 * h : 128 * h + 128] if w == 0
                        else scat[:, 128 * h : 128 * h + 128])
                nc.tensor.matmul(
                    out=IMG[pol][w][h][:],
                    lhsT=lhsT,
                    rhs=rhs,
                    start=False,
                    stop=False,
                )

    def emit_pol_loop(pol):
        seg0 = pol * PC
        for b in range(0, PC, CB):
            span = min(CB, PC - b)
            with tc.For_i(0, span // U,
                          hint_engines=(mybir.EngineType.PE,)) as i:
                for u in range(U):
                    chunk_body(pol, seg0 + b, span, i, u)

    emit_pol_loop(0)
    # neg-segment prep overlaps the pos loop (DVE has idle share there)
    emit_prep(2)
    emit_prep(3)
    raw_cm.__exit__(None, None, None)
    emit_pol_loop(1)

    for p_ in (0, 1):
        for w in (0, 1):
            for h in (0, 1):
                nc.tensor.matmul(out=IMG[p_][w][h][:], lhsT=zl, rhs=zr,
                                 start=False, stop=True)

    loop_pool_cm.__exit__(None, None, None)

    # ---- epilogue ----
    epi_pool = stk.enter_context(tc.tile_pool(name="epi", bufs=1))
    rows = epi_pool.tile([P, 4], F32)
    den = epi_pool.tile([P, 256], F32, tag="den")
    num = epi_pool.tile([P, 256], F32, tag="num")
    rec = epi_pool.tile([P, 256], F32, tag="rec")
    for w in (0, 1):
        SQ = epi_pool.tile([P, 256], F32, tag=f"SQ{w}", name=f"SQ{w}")
        Z = epi_pool.tile([P, 256], F32, tag=f"Z{w}", name=f"Z{w}")
        nc.vector.memset(SQ, 0.0)
        nc.vector.memset(Z, 0.0)
        for h in (0, 1):
            Ph, Nh = IMG[0][w][h], IMG[1][w][h]
            for img in (Ph, Nh):
                nc.vector.tensor_scalar(den, img[:, 0:256], EPS, None, AL.add)
                nc.vector.reciprocal(rec, den)
                nc.vector.tensor_tensor(out=num, in0=img[:, 256:512], in1=rec,
                                        op=AL.mult)
                nc.vector.tensor_tensor(out=num, in0=num, in1=num, op=AL.mult)
                nc.vector.tensor_tensor(out=SQ, in0=SQ, in1=num, op=AL.add)
            # nonzero-pixel count uses den_pos + den_neg
            # (only one tensor_tensor input may come from PSUM -> stage N)
            nc.vector.tensor_copy(out=rec, in_=Nh[:, 0:256])
            nc.vector.tensor_tensor(out=den, in0=Ph[:, 0:256], in1=rec, op=AL.add)
            nc.vector.tensor_scalar(den, den, 0.0, None, AL.is_equal)
            nc.vector.tensor_tensor(out=Z, in0=Z, in1=den, op=AL.add)
        nc.vector.tensor_reduce(
            out=rows[:, 2 * w : 2 * w + 1], in_=SQ, axis=mybir.AxisListType.X,
            op=AL.add,
        )
        nc.vector.tensor_reduce(
            out=rows[:, 2 * w + 1 : 2 * w + 2], in_=Z,
            axis=mybir.AxisListType.X, op=AL.add,
        )

    psum_pool.__exit__(None, None, None)

    with tc.tile_pool(name="psum2", bufs=1, space="PSUM") as psum2:
        red = psum2.tile([1, 4], F32)
        nc.tensor.matmul(out=red[:], lhsT=ones[:], rhs=rows[:], start=True,
                         stop=True)
        scal = epi_pool.tile([1, 4], F32)
        nc.vector.tensor_copy(out=scal, in_=red[:])

    lt = epi_pool.tile([1, 1], F32)
    nc.vector.memset(lt, 0.0)
    t1 = epi_pool.tile([1, 1], F32)
    t2 = epi_pool.tile([1, 1], F32)
    for w in (0, 1):
        # t1 = 65536 - zero_count (the reference's +EPS is an f32 no-op here)
        nc.vector.tensor_scalar(
            t1, scal[0:1, 2 * w + 1 : 2 * w + 2], -1.0, float(NPIX), AL.mult,
            AL.add,
        )
        nc.vector.reciprocal(t2, t1)
        nc.vector.tensor_scalar(
            t1, scal[0:1, 2 * w : 2 * w + 1], 1.0 / (mt * mt), None, AL.mult
        )
        nc.vector.scalar_tensor_tensor(lt, t1, t2, lt, AL.mult, AL.add)

    # Charbonnier temporal-smoothness on vector_list
    d24 = epi_pool.tile([1, 24], F32)
    nc.vector.tensor_tensor(
        out=d24, in0=vtile[0:1, 0:24], in1=vtile[0:1, 8:32], op=AL.subtract
    )
    epsb = epi_pool.tile([1, 1], F32)
    nc.vector.memset(epsb, EPS)
    nc.scalar.activation(d24, d24, ACTF.Square)
    nc.scalar.activation(d24, d24, ACTF.Sqrt, bias=epsb[0:1, 0:1])
    ch = epi_pool.tile([1, 1], F32)
    nc.vector.tensor_reduce(out=ch, in_=d24, axis=mybir.AxisListType.X, op=AL.add)
    nc.vector.scalar_tensor_tensor(lt, ch, FLOW_TEMP_REG / 24.0, lt, AL.mult,
                                   AL.add)

    nc.sync.dma_start(loss_out, lt[:])
    stk.close()


def _build(C2, mt, num_devices=8):
    nc = bacc.Bacc(
        "TRN2", target_bir_lowering=False, debug=False, num_devices=num_devices
    )
    N2 = C2 * P
    ev = nc.dram_tensor("ev", [5, N2], F32, kind="ExternalInput")
    iotas = nc.dram_tensor("iotas", [P, 512], BF16, kind="ExternalInput")
    vecb = nc.dram_tensor("vecb", [1, 32], F32, kind="ExternalInput")
    loss = nc.dram_tensor("loss", [1, 1], F32, kind="ExternalOutput")
    with TileContext(nc) as tc:
        _emit(tc, ev.ap(), iotas.ap(), vecb.ap(), loss.ap(), C2, mt)
    nc.compile()
    return nc


def _host_iotas():
    a = np.arange(256, dtype=np.float32)
    io = np.concatenate([a, 2.0 - a])
    return np.tile(io[None, :], (P, 1)).astype(ml_dtypes.bfloat16)


def _pack_inputs(event_list, flow, pol_mask):
    """Polarity-partition each batch's events, pad each segment to SEGE
    events, lay out as [5, N2] with field matrices [128, C2] flattened
    row-major (chunk = column)."""
    B, N, _ = event_list.shape
    pos_masks = [pol_mask[b, :, 0] > 0.5 for b in range(B)]
    counts = [int(m.sum()) for m in pos_masks]
    maxseg = max(max(c for c in counts), max(N - c for c in counts))
    SEGE = -(-maxseg // (P * U)) * (P * U)  # multiple of P*U events
    PC = SEGE // P
    C2 = 2 * PC

    iot = _host_iotas()
    maps = []
    for b in range(B):
        m = pos_masks[b]
        fields = np.empty((5, P, C2), np.float32)
        ev5 = np.stack([
            event_list[b, :, 0], event_list[b, :, 1], event_list[b, :, 2],
            flow[b, :, 0], flow[b, :, 1],
        ])  # [5, N] (ts, y, x, fy, fx)
        for seg, sel in ((0, m), (1, ~m)):
            data = ev5[:, sel]  # [5, n]
            n = data.shape[1]
            pad = np.zeros((5, SEGE - n), np.float32)
            pad[1:3, :] = PAD_POS  # y, x out of bounds; ts=0, flow=0
            segdata = np.concatenate([data, pad], axis=1)  # [5, SEGE]
            fields[:, :, seg * PC : (seg + 1) * PC] = segdata.reshape(5, P, PC)
        ev_flat = np.ascontiguousarray(fields.reshape(5, P * C2))
        maps.append({"ev": ev_flat, "iotas": iot})
    return maps, C2


_NC_CACHE = {}
LAST_RESULT = None  # BassKernelResults of the most recent run (for test.py)


def kernel(event_list, flow, pol_mask, vector_list, max_ts):
    from concourse.bass_utils import run_bass_kernel_spmd

    event_list = np.asarray(event_list)
    flow = np.asarray(flow)
    pol_mask = np.asarray(pol_mask)
    vector_list = np.asarray(vector_list)
    B, N, _ = event_list.shape
    mt = float(np.asarray(max_ts))

    in_maps, C2 = _pack_inputs(event_list, flow, pol_mask)
    for b in range(B):
        in_maps[b]["vecb"] = np.ascontiguousarray(
            vector_list[b].reshape(1, 32), dtype=np.float32
        )

    key = (C2, mt, B)
    nc = _NC_CACHE.get(key)
    if nc is None:
        nc = _build(C2, mt, num_devices=B)
        _NC_CACHE[key] = nc

    res = run_bass_kernel_spmd(nc, in_maps, core_ids=list(range(B)))
    global LAST_RESULT
    LAST_RESULT = res
    vals = np.array(
        [res.results[b]["loss"][0, 0] for b in range(B)], dtype=np.float32
    )
    return np.float32(np.sum(vals, dtype=np.float32))


# revision 72
# speedup vs baseline: 1.0104x; 1.0104x over previous
"""Trainium2 Bass kernel for nn_EventWarping (contrast-maximization event
warping loss).

Strategy (data-parallel over batch, one NeuronCore per batch element):
  Bilinear scatter-add of N=262144 warped events into 256x256 images via the
  TensorEngine outer-product histogram: per chunk of 128 events,
    image[y, (x|x*ts)] += ty[e, y]^T @ rhs[e, (x|x*ts)]
  where ty is the 2-tap y-tent and rhs packs the x-tent and its ts-weighted
  copy.  Events are polarity-sorted on the host (scatter-add is permutation
  invariant), so each chunk touches only its polarity's 4 PSUM banks,
  halving the matmul count vs an unsorted kernel.
  Per chunk, one GpSimd local_scatter builds warp-1's y-tent, warp-0's rhs
  and warp-1's den half into the first 1024 columns of a [128,1280] tile
  from precomputed int16 indices and bf16 data (8 taps per event,
  out-of-bounds taps mapped to negative indices which local_scatter drops).
  local_scatter's cost is dominated by zeroing its prefix, so warp-1's num
  half [1024:1280] is instead fully overwritten on DVE as den1 * tsw, and
  warp-0's y-tent is built in parallel as relu(min(1-d, 1+d)) with two DVE
  ops (sharing one per-event pointer scalar) and one ACT relu.  The PE runs
  4 (ldweights+matmul) pairs per chunk into 8 PSUM banks.  24 chunks per
  hardware-loop iteration amortize the ~2us all-engine back-edge barrier
  (loop pool bufs=1: the barrier already serializes iterations).
  Field prep (warp positions, floors, tap indices/weights) runs as four
  quarter-width DVE passes: the pos-segment quarters before the pos loop,
  the neg-segment quarters emitted between the loops so they overlap the
  pos loop's Pool/PE time.
  Epilogue computes sum((num/(den+eps))^2)/mt^2/nonzero_px per warp plus the
  Charbonnier flow-smoothness term on device; host sums the 8 per-core
  losses.
"""

import sys

if "/opt/trn_rl_repo" not in sys.path:
    sys.path.insert(0, "/opt/trn_rl_repo")

from contextlib import ExitStack

import ml_dtypes
import numpy as np

import concourse.bacc as bacc
import concourse.bass as bass
import concourse.mybir as mybir
from concourse.tile import TileContext

F32 = mybir.dt.float32
BF16 = mybir.dt.bfloat16
I16 = mybir.dt.int16
I32 = mybir.dt.int32
AL = mybir.AluOpType
ACTF = mybir.ActivationFunctionType

P = 128
RES = 256
NPIX = RES * RES
EPS = 1e-9
FLOW_TEMP_REG = 1e-3
PAD_POS = -1e4  # pad-event coordinate: far out of bounds, zero contribution

NT = 8    # scatter taps: 2 y-taps (warp1) + 4 x-taps (warp0) + 2 den taps (warp1)
SW = 1280  # scatter tile width: ty1[256] rhs0[512] den1[256] num1[256]
SZ = 1024  # zero+scatter region; num1 [1024:1280] is DVE-built (den1 * tsw)
U = 24    # chunks per hw-loop iteration
CB = 72   # chunks per block (dynamic-offset register range: 2*24*16B < 2KB)


def _emit(tc, ev, iotas, vecb, loss_out, C2, mt):
    """C2 = total chunks (pos segment PC + neg segment PC, PC = C2//2)."""
    nc = tc.nc
    PC = C2 // 2
    stk = ExitStack()

    const_pool = stk.enter_context(tc.tile_pool(name="const", bufs=1))
    iota = const_pool.tile([P, 256], BF16)
    nc.sync.dma_start(iota, iotas[:, 0:256])
    iota_n = const_pool.tile([P, 256], BF16)
    nc.sync.dma_start(iota_n, iotas[:, 256:512])
    ones = const_pool.tile([P, 1], F32)
    nc.gpsimd.memset(ones, 1.0)
    zk = const_pool.tile([1, 640], BF16)
    nc.gpsimd.memset(zk, 0.0)
    vtile = const_pool.tile([1, 32], F32)
    nc.sync.dma_start(vtile, vecb)

    # persistent per-event scatter operands: NT taps per event, interleaved
    # per chunk column: idxw/datw[p, NT*c + k], plus warp0 y-tent ptr scalars
    pers_pool = stk.enter_context(tc.tile_pool(name="pers", bufs=1))
    idxw = pers_pool.tile([P, NT * C2], I16)
    datw = pers_pool.tile([P, NT * C2], BF16)
    wy1m0 = pers_pool.tile([P, C2], F32)  # 1 - wy (warp0)
    tsw1 = pers_pool.tile([P, C2], F32)   # mt - ts (warp1 num weight)
    idx4 = idxw[:, 0 : NT * C2].rearrange("p (c f) -> p c f", f=NT)
    dat4 = datw[:, 0 : NT * C2].rearrange("p (c f) -> p c f", f=NT)

    psum_pool = tc.tile_pool(name="psum", bufs=1, space="PSUM")
    psum = psum_pool.__enter__()
    # IMG[pol][w][h]: den in cols 0:256, num in cols 256:512
    IMG = [
        [
            [psum.tile([P, 512], F32, tag=f"I{p_}{w}{h}", name=f"I{p_}{w}{h}")
             for h in (0, 1)]
            for w in (0, 1)
        ]
        for p_ in (0, 1)
    ]

    zl = zk[0:1, 0:128]
    zr = zk[0:1, 128:640]
    for p_ in (0, 1):
        for w in (0, 1):
            for h in (0, 1):
                nc.tensor.matmul(out=IMG[p_][w][h][:], lhsT=zl, rhs=zr,
                                 start=True, stop=False)

    # bufs=1: the default back-edge all-engine barrier already serializes
    # iterations, so cross-iteration double buffering buys nothing; the 24
    # per-u tile sets provide all the within-iteration pipelining.
    loop_pool_cm = tc.tile_pool(name="loop", bufs=1)
    loop_pool = loop_pool_cm.__enter__()

    # ---- prep: [P, CH]-wide field math on DVE, in four quarter passes.
    # Quarters 0-1 (pos segment) run before the pos loop; quarters 2-3 (neg
    # segment) are emitted after it so their DVE work hides under the pos
    # loop's Pool/PE time.  The raw pool stays open across the pos loop
    # (quarter-width tiles keep the combined SBUF footprint under budget).
    CH = C2 // 4
    raw_cm = tc.tile_pool(name="raw", bufs=1)
    raw_pool = raw_cm.__enter__()
    ts_t = raw_pool.tile([P, CH], F32, tag="rts", name="rts")
    y_t = raw_pool.tile([P, CH], F32, tag="ry", name="ry")
    x_t = raw_pool.tile([P, CH], F32, tag="rx", name="rx")
    fy_t = raw_pool.tile([P, CH], F32, tag="rfy", name="rfy")
    fx_t = raw_pool.tile([P, CH], F32, tag="rfx", name="rfx")
    d0 = raw_pool.tile([P, CH], F32, tag="d0")
    scr = raw_pool.tile([P, CH], F32, tag="scr")
    scr2 = raw_pool.tile([P, CH], F32, tag="scr2")
    scri = raw_pool.tile([P, CH], I32, tag="scri")
    wp = [[raw_pool.tile([P, CH], F32, tag=f"wp{w}{a}", name=f"wp{w}{a}")
           for a in (0, 1)] for w in (0, 1)]
    # aliases for x-prep scratch (fields are dead by then within a pass)
    flr = y_t
    f1 = fy_t
    m0 = fx_t
    m1 = x_t

    def emit_prep(q):
        lo = q * CH
        idh = idx4[:, lo : lo + CH, :]
        dah = dat4[:, lo : lo + CH, :]
        for f, t in enumerate((ts_t, y_t, x_t, fy_t, fx_t)):
            src = ev[f : f + 1, :].rearrange("o (p c) -> (o p) c", p=P)
            nc.sync.dma_start(t, src[:, lo : lo + CH])

        nc.vector.tensor_scalar(d0, ts_t, -1.0, float(mt), AL.mult, AL.add)
        for w, tw in ((0, d0), (1, ts_t)):
            op = AL.add if w == 0 else AL.subtract
            nc.vector.tensor_tensor(out=scr, in0=tw, in1=fy_t, op=AL.mult)
            nc.vector.tensor_tensor(out=wp[w][0], in0=y_t, in1=scr, op=op)
            nc.vector.tensor_tensor(out=scr, in0=tw, in1=fx_t, op=AL.mult)
            nc.vector.tensor_tensor(out=wp[w][1], in0=x_t, in1=scr, op=op)

        # warp0 y-tent ptr scalar; warp1 num weight
        nc.vector.tensor_scalar(wy1m0[:, lo : lo + CH], wp[0][0], -1.0, 1.0,
                                AL.mult, AL.add)
        nc.vector.tensor_copy(out=tsw1[:, lo : lo + CH], in_=d0)

        # tap layout per event (offsets into the [P,SW] scatter tile):
        #  0: y0   w1 (off 0)      1: y0+1 w1
        #  2: x0   w0 den (256)    3: x0+1 w0 den
        #  4: x0   w0 num (512)    5: x0+1 w0 num
        #  6: x0   w1 den (768)    7: x0+1 w1 den
        # (w1 num [1024:1280] = den1 * tsw1, built on DVE in the loop)
        for w in (0, 1):
            tw = ts_t if w == 0 else d0
            for a in (0, 1):
                if a == 0 and w == 0:
                    continue  # warp0 y-tent built on DVE/ACT in the loop
                xc = wp[w][a]
                nc.vector.tensor_scalar(xc, xc, 300.0, -4.0, AL.min, AL.max)
                nc.vector.tensor_copy(out=scri, in_=xc)
                nc.vector.tensor_copy(out=flr, in_=scri)
                nc.vector.tensor_tensor(out=scr, in0=flr, in1=xc, op=AL.is_gt)
                nc.vector.tensor_tensor(out=flr, in0=flr, in1=scr,
                                        op=AL.subtract)
                nc.vector.tensor_tensor(out=f1, in0=xc, in1=flr,
                                        op=AL.subtract)
                # masks: m_k = (flr+k <= 255) & (flr+k >= 0 when offset > 0)
                nc.vector.tensor_scalar(m0, flr, 255.5, None, AL.is_le)
                nc.vector.tensor_scalar(m1, flr, 254.5, None, AL.is_le)
                if a == 0:
                    offs = [0]
                elif w == 0:
                    offs = [256, 512]
                else:
                    offs = [768]
                need_lower = offs != [0]
                if need_lower:
                    nc.vector.tensor_scalar(scr2, flr, -0.5, None, AL.is_ge)
                    nc.vector.tensor_tensor(out=m0, in0=m0, in1=scr2,
                                            op=AL.mult)
                    nc.vector.tensor_scalar(scr2, flr, -1.5, None, AL.is_ge)
                    nc.vector.tensor_tensor(out=m1, in0=m1, in1=scr2,
                                            op=AL.mult)
                base_k = 0 if a == 0 else (2 + 4 * w)
                for j, off in enumerate(offs):
                    # i0 = (flr+off+1)*m0 - 1 ; i1 = (flr+off+2)*m1 - 1
                    nc.vector.tensor_scalar(scr, flr, float(off + 1), None,
                                            AL.add)
                    nc.vector.tensor_tensor(out=scr, in0=scr, in1=m0,
                                            op=AL.mult)
                    nc.vector.tensor_scalar(scr, scr, -1.0, None, AL.add)
                    nc.vector.tensor_copy(out=idh[:, :, base_k + 2 * j],
                                          in_=scr)
                    nc.vector.tensor_scalar(scr, flr, float(off + 2), None,
                                            AL.add)
                    nc.vector.tensor_tensor(out=scr, in0=scr, in1=m1,
                                            op=AL.mult)
                    nc.vector.tensor_scalar(scr, scr, -1.0, None, AL.add)
                    nc.vector.tensor_copy(out=idh[:, :, base_k + 2 * j + 1],
                                          in_=scr)
                # data: f0 = 1-f1 at tap0, f1 at tap1 (den); *tw for num taps
                nc.vector.tensor_scalar(scr, f1, -1.0, 1.0, AL.mult, AL.add)
                nc.vector.tensor_copy(out=dah[:, :, base_k], in_=scr)
                nc.vector.tensor_copy(out=dah[:, :, base_k + 1], in_=f1)
                if a == 1 and w == 0:
                    nc.vector.tensor_tensor(out=scr, in0=scr, in1=tw,
                                            op=AL.mult)
                    nc.vector.tensor_copy(out=dah[:, :, base_k + 2], in_=scr)
                    nc.vector.tensor_tensor(out=scr, in0=f1, in1=tw,
                                            op=AL.mult)
                    nc.vector.tensor_copy(out=dah[:, :, base_k + 3], in_=scr)

    emit_prep(0)
    emit_prep(1)

    def chunk_body(pol, base, span, i, u):
        iv = idx4[:, base + u : base + span : U, :][:, bass.ds(i, 1), :]
        dv = dat4[:, base + u : base + span : U, :][:, bass.ds(i, 1), :]
        scat = loop_pool.tile([P, SW], BF16, tag=f"s{u}", name=f"s{u}")
        nc.gpsimd.local_scatter(scat[:, 0:SZ], dv, iv, channels=P,
                                num_elems=SZ, num_idxs=NT)
        # warp1 num half: den1 * tsw1 (fully overwrites [1024:1280])
        t1c = tsw1[:, base + u : base + span : U][:, bass.ds(i, 1)]
        nc.vector.tensor_scalar(scat[:, 1024:1280], scat[:, 768:1024], t1c,
                                None, AL.mult)
        # warp0 y-tent: ut = 1+d, tp = min(1-d, 1+d), ty0 = relu(tp)
        # (iota_n holds 2-iota so both ops share the single wy1m scalar)
        w1c = wy1m0[:, base + u : base + span : U][:, bass.ds(i, 1)]
        ut = loop_pool.tile([P, 256], BF16, tag=f"u{u}", name=f"u{u}")
        nc.vector.tensor_scalar(ut, iota, w1c, None, AL.add)
        nc.vector.scalar_tensor_tensor(ut, iota_n, w1c, ut, AL.subtract, AL.min)
        ty0 = loop_pool.tile([P, 256], BF16, tag=f"t{u}", name=f"t{u}")
        nc.scalar.activation(ty0, ut, ACTF.Relu)
        for w in (0, 1):
            rhs = scat[:, 256 + 512 * w : 768 + 512 * w]
            for h in (0, 1):
                lhsT = (ty0[:, 128 * h : 128 * h + 128] if w == 0
                        else scat[:, 128 * h : 128 * h + 128])
                nc.tensor.matmul(
                    out=IMG[pol][w][h][:],
                    lhsT=lhsT,
                    rhs=rhs,
                    start=False,
                    stop=False,
                )

    def emit_pol_loop(pol):
        seg0 = pol * PC
        for b in range(0, PC, CB):
            span = min(CB, PC - b)
            with tc.For_i(0, span // U,
                          hint_engines=(mybir.EngineType.PE,)) as i:
                for u in range(U):
                    chunk_body(pol, seg0 + b, span, i, u)

    emit_pol_loop(0)
    # neg-segment prep overlaps the pos loop (DVE has idle share there)
    emit_prep(2)
    emit_prep(3)
    raw_cm.__exit__(None, None, None)
    emit_pol_loop(1)

    for p_ in (0, 1):
        for w in (0, 1):
            for h in (0, 1):
                nc.tensor.matmul(out=IMG[p_][w][h][:], lhsT=zl, rhs=zr,
                                 start=False, stop=True)

    loop_pool_cm.__exit__(None, None, None)

    # ---- epilogue ----
    epi_pool = stk.enter_context(tc.tile_pool(name="epi", bufs=1))
    rows = epi_pool.tile([P, 4], F32)
    den = epi_pool.tile([P, 256], F32, tag="den")
    num = epi_pool.tile([P, 256], F32, tag="num")
    rec = epi_pool.tile([P, 256], F32, tag="rec")
    for w in (0, 1):
        SQ = epi_pool.tile([P, 256], F32, tag=f"SQ{w}", name=f"SQ{w}")
        Z = epi_pool.tile([P, 256], F32, tag=f"Z{w}", name=f"Z{w}")
        nc.vector.memset(SQ, 0.0)
        nc.vector.memset(Z, 0.0)
        for h in (0, 1):
            Ph, Nh = IMG[0][w][h], IMG[1][w][h]
            for img in (Ph, Nh):
                nc.vector.tensor_scalar(den, img[:, 0:256], EPS, None, AL.add)
                nc.vector.reciprocal(rec, den)
                nc.vector.tensor_tensor(out=num, in0=img[:, 256:512], in1=rec,
                                        op=AL.mult)
                nc.vector.tensor_tensor(out=num, in0=num, in1=num, op=AL.mult)
                nc.vector.tensor_tensor(out=SQ, in0=SQ, in1=num, op=AL.add)
            # nonzero-pixel count uses den_pos + den_neg
            # (only one tensor_tensor input may come from PSUM -> stage N)
            nc.vector.tensor_copy(out=rec, in_=Nh[:, 0:256])
            nc.vector.tensor_tensor(out=den, in0=Ph[:, 0:256], in1=rec, op=AL.add)
            nc.vector.tensor_scalar(den, den, 0.0, None, AL.is_equal)
            nc.vector.tensor_tensor(out=Z, in0=Z, in1=den, op=AL.add)
        nc.vector.tensor_reduce(
            out=rows[:, 2 * w : 2 * w + 1], in_=SQ, axis=mybir.AxisListType.X,
            op=AL.add,
        )
        nc.vector.tensor_reduce(
            out=rows[:, 2 * w + 1 : 2 * w + 2], in_=Z,
            axis=mybir.AxisListType.X, op=AL.add,
        )

    psum_pool.__exit__(None, None, None)

    with tc.tile_pool(name="psum2", bufs=1, space="PSUM") as psum2:
        red = psum2.tile([1, 4], F32)
        nc.tensor.matmul(out=red[:], lhsT=ones[:], rhs=rows[:], start=True,
                         stop=True)
        scal = epi_pool.tile([1, 4], F32)
        nc.vector.tensor_copy(out=scal, in_=red[:])

    lt = epi_pool.tile([1, 1], F32)
    nc.vector.memset(lt, 0.0)
    t1 = epi_pool.tile([1, 1], F32)
    t2 = epi_pool.tile([1, 1], F32)
    for w in (0, 1):
        # t1 = 65536 - zero_count (the reference's +EPS is an f32 no-op here)
        nc.vector.tensor_scalar(
            t1, scal[0:1, 2 * w + 1 : 2 * w + 2], -1.0, float(NPIX), AL.mult,
            AL.add,
        )
        nc.vector.reciprocal(t2, t1)
        nc.vector.tensor_scalar(
            t1, scal[0:1, 2 * w : 2 * w + 1], 1.0 / (mt * mt), None, AL.mult
        )
        nc.vector.scalar_tensor_tensor(lt, t1, t2, lt, AL.mult, AL.add)

    # Charbonnier temporal-smoothness on vector_list
    d24 = epi_pool.tile([1, 24], F32)
    nc.vector.tensor_tensor(
        out=d24, in0=vtile[0:1, 0:24], in1=vtile[0:1, 8:32], op=AL.subtract
    )
    epsb = epi_pool.tile([1, 1], F32)
    nc.vector.memset(epsb, EPS)
    nc.scalar.activation(d24, d24, ACTF.Square)
    nc.scalar.activation(d24, d24, ACTF.Sqrt, bias=epsb[0:1, 0:1])
    ch = epi_pool.tile([1, 1], F32)
    nc.vector.tensor_reduce(out=ch, in_=d24, axis=mybir.AxisListType.X, op=AL.add)
    nc.vector.scalar_tensor_tensor(lt, ch, FLOW_TEMP_REG / 24.0, lt, AL.mult,
                                   AL.add)

    nc.sync.dma_start(loss_out, lt[:])
    stk.close()


def _build(C2, mt, num_devices=8):
    nc = bacc.Bacc(
        "TRN2", target_bir_lowering=False, debug=False, num_devices=num_devices
    )
    N2 = C2 * P
    ev = nc.dram_tensor("ev", [5, N2], F32, kind="ExternalInput")
    iotas = nc.dram_tensor("iotas", [P, 512], BF16, kind="ExternalInput")
    vecb = nc.dram_tensor("vecb", [1, 32], F32, kind="ExternalInput")
    loss = nc.dram_tensor("loss", [1, 1], F32, kind="ExternalOutput")
    with TileContext(nc) as tc:
        _emit(tc, ev.ap(), iotas.ap(), vecb.ap(), loss.ap(), C2, mt)
    nc.compile()
    return nc


def _host_iotas():
    a = np.arange(256, dtype=np.float32)
    io = np.concatenate([a, 2.0 - a])
    return np.tile(io[None, :], (P, 1)).astype(ml_dtypes.bfloat16)


def _pack_inputs(event_list, flow, pol_mask):
    """Polarity-partition each batch's events, pad each segment to SEGE
    events, lay out as [5, N2] with field matrices [128, C2] flattened
    row-major (chunk = column)."""
    B, N, _ = event_list.shape
    pos_masks = [pol_mask[b, :, 0] > 0.5 for b in range(B)]
    counts = [int(m.sum()) for m in pos_masks]
    maxseg = max(max(c for c in counts), max(N - c for c in counts))
    SEGE = -(-maxseg // (P * U)) * (P * U)  # multiple of P*U events
    PC = SEGE // P
    C2 = 2 * PC

    iot = _host_iotas()
    maps = []
    for b in range(B):
        m = pos_masks[b]
        fields = np.empty((5, P, C2), np.float32)
        ev5 = np.stack([
            event_list[b, :, 0], event_list[b, :, 1], event_list[b, :, 2],
            flow[b, :, 0], flow[b, :, 1],
        ])  # [5, N] (ts, y, x, fy, fx)
        for seg, sel in ((0, m), (1, ~m)):
            data = ev5[:, sel]  # [5, n]
            n = data.shape[1]
            pad = np.zeros((5, SEGE - n), np.float32)
            pad[1:3, :] = PAD_POS  # y, x out of bounds; ts=0, flow=0
            segdata = np.concatenate([data, pad], axis=1)  # [5, SEGE]
            fields[:, :, seg * PC : (seg + 1) * PC] = segdata.reshape(5, P, PC)
        ev_flat = np.ascontiguousarray(fields.reshape(5, P * C2))
        maps.append({"ev": ev_flat, "iotas": iot})
    return maps, C2


_NC_CACHE = {}
LAST_RESULT = None  # BassKernelResults of the most recent run (for test.py)


def kernel(event_list, flow, pol_mask, vector_list, max_ts):
    from concourse.bass_utils import run_bass_kernel_spmd

    event_list = np.asarray(event_list)
    flow = np.asarray(flow)
    pol_mask = np.asarray(pol_mask)
    vector_list = np.asarray(vector_list)
    B, N, _ = event_list.shape
    mt = float(np.asarray(max_ts))

    in_maps, C2 = _pack_inputs(event_list, flow, pol_mask)
    for b in range(B):
        in_maps[b]["vecb"] = np.ascontiguousarray(
            vector_list[b].reshape(1, 32), dtype=np.float32
        )

    key = (C2, mt, B)
    nc = _NC_CACHE.get(key)
    if nc is None:
        nc = _build(C2, mt, num_devices=B)
        _NC_CACHE[key] = nc

    res = run_bass_kernel_spmd(nc, in_maps, core_ids=list(range(B)))
    global LAST_RESULT
    LAST_RESULT = res
    vals = np.array(
        [res.results[b]["loss"][0, 0] for b in range(B)], dtype=np.float32
    )
    return np.float32(np.sum(vals, dtype=np.float32))


# revision 73
# speedup vs baseline: 1.0206x; 1.0101x over previous
"""Trainium2 Bass kernel for nn_EventWarping (contrast-maximization event
warping loss).

Strategy (data-parallel over batch, one NeuronCore per batch element):
  Bilinear scatter-add of N=262144 warped events into 256x256 images via the
  TensorEngine outer-product histogram: per chunk of 128 events,
    image[y, (x|x*ts)] += ty[e, y]^T @ rhs[e, (x|x*ts)]
  where ty is the 2-tap y-tent and rhs packs the x-tent and its ts-weighted
  copy.  Events are polarity-sorted on the host (scatter-add is permutation
  invariant), so each chunk touches only its polarity's 4 PSUM banks,
  halving the matmul count vs an unsorted kernel.
  Per chunk, one GpSimd local_scatter builds warp-1's y-tent, warp-0's rhs
  and warp-1's den half into the first 1024 columns of a [128,1280] tile
  from precomputed int16 indices and bf16 data (8 taps per event,
  out-of-bounds taps mapped to negative indices which local_scatter drops).
  local_scatter's cost is dominated by zeroing its prefix, so warp-1's num
  half [1024:1280] is instead fully overwritten on DVE as den1 * tsw, and
  warp-0's y-tent is built in parallel as relu(min(1-d, 1+d)) with two DVE
  ops (sharing one per-event pointer scalar) and one ACT relu.  The PE runs
  4 (ldweights+matmul) pairs per chunk into 8 PSUM banks.  24 chunks per
  hardware-loop iteration amortize the ~2us all-engine back-edge barrier
  (loop pool bufs=1: the barrier already serializes iterations).
  Field prep (warp positions, floors, tap indices/weights) runs as four
  quarter-width DVE passes: the pos-segment quarters before the pos loop,
  the neg-segment quarters emitted between the loops so they overlap the
  pos loop's Pool/PE time.
  Epilogue computes sum((num/(den+eps))^2)/mt^2/nonzero_px per warp plus the
  Charbonnier flow-smoothness term on device; host sums the 8 per-core
  losses.
"""

import sys

if "/opt/trn_rl_repo" not in sys.path:
    sys.path.insert(0, "/opt/trn_rl_repo")

from contextlib import ExitStack

import ml_dtypes
import numpy as np

import concourse.bacc as bacc
import concourse.bass as bass
import concourse.mybir as mybir
from concourse.tile import TileContext

F32 = mybir.dt.float32
BF16 = mybir.dt.bfloat16
I16 = mybir.dt.int16
I32 = mybir.dt.int32
AL = mybir.AluOpType
ACTF = mybir.ActivationFunctionType

P = 128
RES = 256
NPIX = RES * RES
EPS = 1e-9
FLOW_TEMP_REG = 1e-3
PAD_POS = -1e4  # pad-event coordinate: far out of bounds, zero contribution

NT = 8    # scatter taps: 2 y-taps (warp1) + 4 x-taps (warp0) + 2 den taps (warp1)
SW = 1280  # scatter tile width: ty1[256] rhs0[512] den1[256] num1[256]
SZ = 1024  # zero+scatter region; num1 [1024:1280] is DVE-built (den1 * tsw)
U = 28    # chunks per hw-loop iteration (PE body 224 instrs < 256 IRAM limit)
CB = 84   # chunks per block (dynamic-offset register range: 2*28*16B < 2KB)


def _emit(tc, ev, iotas, vecb, loss_out, C2, mt):
    """C2 = total chunks (pos segment PC + neg segment PC, PC = C2//2)."""
    nc = tc.nc
    PC = C2 // 2
    stk = ExitStack()

    const_pool = stk.enter_context(tc.tile_pool(name="const", bufs=1))
    iota = const_pool.tile([P, 256], BF16)
    nc.sync.dma_start(iota, iotas[:, 0:256])
    iota_n = const_pool.tile([P, 256], BF16)
    nc.sync.dma_start(iota_n, iotas[:, 256:512])
    ones = const_pool.tile([P, 1], F32)
    nc.gpsimd.memset(ones, 1.0)
    zk = const_pool.tile([1, 640], BF16)
    nc.gpsimd.memset(zk, 0.0)
    vtile = const_pool.tile([1, 32], F32)
    nc.sync.dma_start(vtile, vecb)

    # persistent per-event scatter operands: NT taps per event, interleaved
    # per chunk column: idxw/datw[p, NT*c + k], plus warp0 y-tent ptr scalars
    pers_pool = stk.enter_context(tc.tile_pool(name="pers", bufs=1))
    idxw = pers_pool.tile([P, NT * C2], I16)
    datw = pers_pool.tile([P, NT * C2], BF16)
    wy1m0 = pers_pool.tile([P, C2], F32)  # 1 - wy (warp0)
    tsw1 = pers_pool.tile([P, C2], F32)   # mt - ts (warp1 num weight)
    idx4 = idxw[:, 0 : NT * C2].rearrange("p (c f) -> p c f", f=NT)
    dat4 = datw[:, 0 : NT * C2].rearrange("p (c f) -> p c f", f=NT)

    psum_pool = tc.tile_pool(name="psum", bufs=1, space="PSUM")
    psum = psum_pool.__enter__()
    # IMG[pol][w][h]: den in cols 0:256, num in cols 256:512
    IMG = [
        [
            [psum.tile([P, 512], F32, tag=f"I{p_}{w}{h}", name=f"I{p_}{w}{h}")
             for h in (0, 1)]
            for w in (0, 1)
        ]
        for p_ in (0, 1)
    ]

    zl = zk[0:1, 0:128]
    zr = zk[0:1, 128:640]
    for p_ in (0, 1):
        for w in (0, 1):
            for h in (0, 1):
                nc.tensor.matmul(out=IMG[p_][w][h][:], lhsT=zl, rhs=zr,
                                 start=True, stop=False)

    # bufs=1: the default back-edge all-engine barrier already serializes
    # iterations, so cross-iteration double buffering buys nothing; the 24
    # per-u tile sets provide all the within-iteration pipelining.
    loop_pool_cm = tc.tile_pool(name="loop", bufs=1)
    loop_pool = loop_pool_cm.__enter__()

    # ---- prep: [P, CH]-wide field math on DVE, in four quarter passes.
    # Quarters 0-1 (pos segment) run before the pos loop; quarters 2-3 (neg
    # segment) are emitted after it so their DVE work hides under the pos
    # loop's Pool/PE time.  The raw pool stays open across the pos loop
    # (quarter-width tiles keep the combined SBUF footprint under budget).
    CH = C2 // 4
    raw_cm = tc.tile_pool(name="raw", bufs=1)
    raw_pool = raw_cm.__enter__()
    ts_t = raw_pool.tile([P, CH], F32, tag="rts", name="rts")
    y_t = raw_pool.tile([P, CH], F32, tag="ry", name="ry")
    x_t = raw_pool.tile([P, CH], F32, tag="rx", name="rx")
    fy_t = raw_pool.tile([P, CH], F32, tag="rfy", name="rfy")
    fx_t = raw_pool.tile([P, CH], F32, tag="rfx", name="rfx")
    d0 = raw_pool.tile([P, CH], F32, tag="d0")
    scr = raw_pool.tile([P, CH], F32, tag="scr")
    scr2 = raw_pool.tile([P, CH], F32, tag="scr2")
    scri = raw_pool.tile([P, CH], I32, tag="scri")
    wp = [[raw_pool.tile([P, CH], F32, tag=f"wp{w}{a}", name=f"wp{w}{a}")
           for a in (0, 1)] for w in (0, 1)]
    # aliases for x-prep scratch (fields are dead by then within a pass)
    flr = y_t
    f1 = fy_t
    m0 = fx_t
    m1 = x_t

    def emit_prep(q):
        lo = q * CH
        idh = idx4[:, lo : lo + CH, :]
        dah = dat4[:, lo : lo + CH, :]
        for f, t in enumerate((ts_t, y_t, x_t, fy_t, fx_t)):
            src = ev[f : f + 1, :].rearrange("o (p c) -> (o p) c", p=P)
            nc.sync.dma_start(t, src[:, lo : lo + CH])

        nc.vector.tensor_scalar(d0, ts_t, -1.0, float(mt), AL.mult, AL.add)
        for w, tw in ((0, d0), (1, ts_t)):
            op = AL.add if w == 0 else AL.subtract
            nc.vector.tensor_tensor(out=scr, in0=tw, in1=fy_t, op=AL.mult)
            nc.vector.tensor_tensor(out=wp[w][0], in0=y_t, in1=scr, op=op)
            nc.vector.tensor_tensor(out=scr, in0=tw, in1=fx_t, op=AL.mult)
            nc.vector.tensor_tensor(out=wp[w][1], in0=x_t, in1=scr, op=op)

        # warp0 y-tent ptr scalar; warp1 num weight
        nc.vector.tensor_scalar(wy1m0[:, lo : lo + CH], wp[0][0], -1.0, 1.0,
                                AL.mult, AL.add)
        nc.vector.tensor_copy(out=tsw1[:, lo : lo + CH], in_=d0)

        # tap layout per event (offsets into the [P,SW] scatter tile):
        #  0: y0   w1 (off 0)      1: y0+1 w1
        #  2: x0   w0 den (256)    3: x0+1 w0 den
        #  4: x0   w0 num (512)    5: x0+1 w0 num
        #  6: x0   w1 den (768)    7: x0+1 w1 den
        # (w1 num [1024:1280] = den1 * tsw1, built on DVE in the loop)
        for w in (0, 1):
            tw = ts_t if w == 0 else d0
            for a in (0, 1):
                if a == 0 and w == 0:
                    continue  # warp0 y-tent built on DVE/ACT in the loop
                xc = wp[w][a]
                nc.vector.tensor_scalar(xc, xc, 300.0, -4.0, AL.min, AL.max)
                nc.vector.tensor_copy(out=scri, in_=xc)
                nc.vector.tensor_copy(out=flr, in_=scri)
                nc.vector.tensor_tensor(out=scr, in0=flr, in1=xc, op=AL.is_gt)
                nc.vector.tensor_tensor(out=flr, in0=flr, in1=scr,
                                        op=AL.subtract)
                nc.vector.tensor_tensor(out=f1, in0=xc, in1=flr,
                                        op=AL.subtract)
                # masks: m_k = (flr+k <= 255) & (flr+k >= 0 when offset > 0)
                nc.vector.tensor_scalar(m0, flr, 255.5, None, AL.is_le)
                nc.vector.tensor_scalar(m1, flr, 254.5, None, AL.is_le)
                if a == 0:
                    offs = [0]
                elif w == 0:
                    offs = [256, 512]
                else:
                    offs = [768]
                need_lower = offs != [0]
                if need_lower:
                    nc.vector.tensor_scalar(scr2, flr, -0.5, None, AL.is_ge)
                    nc.vector.tensor_tensor(out=m0, in0=m0, in1=scr2,
                                            op=AL.mult)
                    nc.vector.tensor_scalar(scr2, flr, -1.5, None, AL.is_ge)
                    nc.vector.tensor_tensor(out=m1, in0=m1, in1=scr2,
                                            op=AL.mult)
                base_k = 0 if a == 0 else (2 + 4 * w)
                for j, off in enumerate(offs):
                    # i0 = (flr+off+1)*m0 - 1 ; i1 = (flr+off+2)*m1 - 1
                    nc.vector.tensor_scalar(scr, flr, float(off + 1), None,
                                            AL.add)
                    nc.vector.tensor_tensor(out=scr, in0=scr, in1=m0,
                                            op=AL.mult)
                    nc.vector.tensor_scalar(scr, scr, -1.0, None, AL.add)
                    nc.vector.tensor_copy(out=idh[:, :, base_k + 2 * j],
                                          in_=scr)
                    nc.vector.tensor_scalar(scr, flr, float(off + 2), None,
                                            AL.add)
                    nc.vector.tensor_tensor(out=scr, in0=scr, in1=m1,
                                            op=AL.mult)
                    nc.vector.tensor_scalar(scr, scr, -1.0, None, AL.add)
                    nc.vector.tensor_copy(out=idh[:, :, base_k + 2 * j + 1],
                                          in_=scr)
                # data: f0 = 1-f1 at tap0, f1 at tap1 (den); *tw for num taps
                nc.vector.tensor_scalar(scr, f1, -1.0, 1.0, AL.mult, AL.add)
                nc.vector.tensor_copy(out=dah[:, :, base_k], in_=scr)
                nc.vector.tensor_copy(out=dah[:, :, base_k + 1], in_=f1)
                if a == 1 and w == 0:
                    nc.vector.tensor_tensor(out=scr, in0=scr, in1=tw,
                                            op=AL.mult)
                    nc.vector.tensor_copy(out=dah[:, :, base_k + 2], in_=scr)
                    nc.vector.tensor_tensor(out=scr, in0=f1, in1=tw,
                                            op=AL.mult)
                    nc.vector.tensor_copy(out=dah[:, :, base_k + 3], in_=scr)

    emit_prep(0)
    emit_prep(1)

    def chunk_body(pol, base, span, i, u):
        iv = idx4[:, base + u : base + span : U, :][:, bass.ds(i, 1), :]
        dv = dat4[:, base + u : base + span : U, :][:, bass.ds(i, 1), :]
        scat = loop_pool.tile([P, SW], BF16, tag=f"s{u}", name=f"s{u}")
        nc.gpsimd.local_scatter(scat[:, 0:SZ], dv, iv, channels=P,
                                num_elems=SZ, num_idxs=NT)
        # warp1 num half: den1 * tsw1 (fully overwrites [1024:1280])
        t1c = tsw1[:, base + u : base + span : U][:, bass.ds(i, 1)]
        nc.vector.tensor_scalar(scat[:, 1024:1280], scat[:, 768:1024], t1c,
                                None, AL.mult)
        # warp0 y-tent: ut = 1+d, tp = min(1-d, 1+d), ty0 = relu(tp)
        # (iota_n holds 2-iota so both ops share the single wy1m scalar)
        w1c = wy1m0[:, base + u : base + span : U][:, bass.ds(i, 1)]
        ut = loop_pool.tile([P, 256], BF16, tag=f"u{u}", name=f"u{u}")
        nc.vector.tensor_scalar(ut, iota, w1c, None, AL.add)
        nc.vector.scalar_tensor_tensor(ut, iota_n, w1c, ut, AL.subtract, AL.min)
        ty0 = loop_pool.tile([P, 256], BF16, tag=f"t{u}", name=f"t{u}")
        nc.scalar.activation(ty0, ut, ACTF.Relu)
        for w in (0, 1):
            rhs = scat[:, 256 + 512 * w : 768 + 512 * w]
            for h in (0, 1):
                lhsT = (ty0[:, 128 * h : 128 * h + 128] if w == 0
                        else scat[:, 128 * h : 128 * h + 128])
                nc.tensor.matmul(
                    out=IMG[pol][w][h][:],
                    lhsT=lhsT,
                    rhs=rhs,
                    start=False,
                    stop=False,
                )

    def emit_pol_loop(pol):
        seg0 = pol * PC
        for b in range(0, PC, CB):
            span = min(CB, PC - b)
            with tc.For_i(0, span // U,
                          hint_engines=(mybir.EngineType.PE,)) as i:
                for u in range(U):
                    chunk_body(pol, seg0 + b, span, i, u)

    emit_pol_loop(0)
    # neg-segment prep overlaps the pos loop (DVE has idle share there)
    emit_prep(2)
    emit_prep(3)
    raw_cm.__exit__(None, None, None)
    emit_pol_loop(1)

    for p_ in (0, 1):
        for w in (0, 1):
            for h in (0, 1):
                nc.tensor.matmul(out=IMG[p_][w][h][:], lhsT=zl, rhs=zr,
                                 start=False, stop=True)

    loop_pool_cm.__exit__(None, None, None)

    # ---- epilogue ----
    epi_pool = stk.enter_context(tc.tile_pool(name="epi", bufs=1))
    rows = epi_pool.tile([P, 4], F32)
    den = epi_pool.tile([P, 256], F32, tag="den")
    num = epi_pool.tile([P, 256], F32, tag="num")
    rec = epi_pool.tile([P, 256], F32, tag="rec")
    for w in (0, 1):
        SQ = epi_pool.tile([P, 256], F32, tag=f"SQ{w}", name=f"SQ{w}")
        Z = epi_pool.tile([P, 256], F32, tag=f"Z{w}", name=f"Z{w}")
        nc.vector.memset(SQ, 0.0)
        nc.vector.memset(Z, 0.0)
        for h in (0, 1):
            Ph, Nh = IMG[0][w][h], IMG[1][w][h]
            for img in (Ph, Nh):
                nc.vector.tensor_scalar(den, img[:, 0:256], EPS, None, AL.add)
                nc.vector.reciprocal(rec, den)
                nc.vector.tensor_tensor(out=num, in0=img[:, 256:512], in1=rec,
                                        op=AL.mult)
                nc.vector.tensor_tensor(out=num, in0=num, in1=num, op=AL.mult)
                nc.vector.tensor_tensor(out=SQ, in0=SQ, in1=num, op=AL.add)
            # nonzero-pixel count uses den_pos + den_neg
            # (only one tensor_tensor input may come from PSUM -> stage N)
            nc.vector.tensor_copy(out=rec, in_=Nh[:, 0:256])
            nc.vector.tensor_tensor(out=den, in0=Ph[:, 0:256], in1=rec, op=AL.add)
            nc.vector.tensor_scalar(den, den, 0.0, None, AL.is_equal)
            nc.vector.tensor_tensor(out=Z, in0=Z, in1=den, op=AL.add)
        nc.vector.tensor_reduce(
            out=rows[:, 2 * w : 2 * w + 1], in_=SQ, axis=mybir.AxisListType.X,
            op=AL.add,
        )
        nc.vector.tensor_reduce(
            out=rows[:, 2 * w + 1 : 2 * w + 2], in_=Z,
            axis=mybir.AxisListType.X, op=AL.add,
        )

    psum_pool.__exit__(None, None, None)

    with tc.tile_pool(name="psum2", bufs=1, space="PSUM") as psum2:
        red = psum2.tile([1, 4], F32)
        nc.tensor.matmul(out=red[:], lhsT=ones[:], rhs=rows[:], start=True,
                         stop=True)
        scal = epi_pool.tile([1, 4], F32)
        nc.vector.tensor_copy(out=scal, in_=red[:])

    lt = epi_pool.tile([1, 1], F32)
    nc.vector.memset(lt, 0.0)
    t1 = epi_pool.tile([1, 1], F32)
    t2 = epi_pool.tile([1, 1], F32)
    for w in (0, 1):
        # t1 = 65536 - zero_count (the reference's +EPS is an f32 no-op here)
        nc.vector.tensor_scalar(
            t1, scal[0:1, 2 * w + 1 : 2 * w + 2], -1.0, float(NPIX), AL.mult,
            AL.add,
        )
        nc.vector.reciprocal(t2, t1)
        nc.vector.tensor_scalar(
            t1, scal[0:1, 2 * w : 2 * w + 1], 1.0 / (mt * mt), None, AL.mult
        )
        nc.vector.scalar_tensor_tensor(lt, t1, t2, lt, AL.mult, AL.add)

    # Charbonnier temporal-smoothness on vector_list
    d24 = epi_pool.tile([1, 24], F32)
    nc.vector.tensor_tensor(
        out=d24, in0=vtile[0:1, 0:24], in1=vtile[0:1, 8:32], op=AL.subtract
    )
    epsb = epi_pool.tile([1, 1], F32)
    nc.vector.memset(epsb, EPS)
    nc.scalar.activation(d24, d24, ACTF.Square)
    nc.scalar.activation(d24, d24, ACTF.Sqrt, bias=epsb[0:1, 0:1])
    ch = epi_pool.tile([1, 1], F32)
    nc.vector.tensor_reduce(out=ch, in_=d24, axis=mybir.AxisListType.X, op=AL.add)
    nc.vector.scalar_tensor_tensor(lt, ch, FLOW_TEMP_REG / 24.0, lt, AL.mult,
                                   AL.add)

    nc.sync.dma_start(loss_out, lt[:])
    stk.close()


def _build(C2, mt, num_devices=8):
    nc = bacc.Bacc(
        "TRN2", target_bir_lowering=False, debug=False, num_devices=num_devices
    )
    N2 = C2 * P
    ev = nc.dram_tensor("ev", [5, N2], F32, kind="ExternalInput")
    iotas = nc.dram_tensor("iotas", [P, 512], BF16, kind="ExternalInput")
    vecb = nc.dram_tensor("vecb", [1, 32], F32, kind="ExternalInput")
    loss = nc.dram_tensor("loss", [1, 1], F32, kind="ExternalOutput")
    with TileContext(nc) as tc:
        _emit(tc, ev.ap(), iotas.ap(), vecb.ap(), loss.ap(), C2, mt)
    nc.compile()
    return nc


def _host_iotas():
    a = np.arange(256, dtype=np.float32)
    io = np.concatenate([a, 2.0 - a])
    return np.tile(io[None, :], (P, 1)).astype(ml_dtypes.bfloat16)


def _pack_inputs(event_list, flow, pol_mask):
    """Polarity-partition each batch's events, pad each segment to SEGE
    events, lay out as [5, N2] with field matrices [128, C2] flattened
    row-major (chunk = column)."""
    B, N, _ = event_list.shape
    pos_masks = [pol_mask[b, :, 0] > 0.5 for b in range(B)]
    counts = [int(m.sum()) for m in pos_masks]
    maxseg = max(max(c for c in counts), max(N - c for c in counts))
    SEGE = -(-maxseg // (P * U)) * (P * U)  # multiple of P*U events
    PC = SEGE // P
    C2 = 2 * PC

    iot = _host_iotas()
    maps = []
    for b in range(B):
        m = pos_masks[b]
        fields = np.empty((5, P, C2), np.float32)
        ev5 = np.stack([
            event_list[b, :, 0], event_list[b, :, 1], event_list[b, :, 2],
            flow[b, :, 0], flow[b, :, 1],
        ])  # [5, N] (ts, y, x, fy, fx)
        for seg, sel in ((0, m), (1, ~m)):
            data = ev5[:, sel]  # [5, n]
            n = data.shape[1]
            pad = np.zeros((5, SEGE - n), np.float32)
            pad[1:3, :] = PAD_POS  # y, x out of bounds; ts=0, flow=0
            segdata = np.concatenate([data, pad], axis=1)  # [5, SEGE]
            fields[:, :, seg * PC : (seg + 1) * PC] = segdata.reshape(5, P, PC)
        ev_flat = np.ascontiguousarray(fields.reshape(5, P * C2))
        maps.append({"ev": ev_flat, "iotas": iot})
    return maps, C2


_NC_CACHE = {}
LAST_RESULT = None  # BassKernelResults of the most recent run (for test.py)


def kernel(event_list, flow, pol_mask, vector_list, max_ts):
    from concourse.bass_utils import run_bass_kernel_spmd

    event_list = np.asarray(event_list)
    flow = np.asarray(flow)
    pol_mask = np.asarray(pol_mask)
    vector_list = np.asarray(vector_list)
    B, N, _ = event_list.shape
    mt = float(np.asarray(max_ts))

    in_maps, C2 = _pack_inputs(event_list, flow, pol_mask)
    for b in range(B):
        in_maps[b]["vecb"] = np.ascontiguousarray(
            vector_list[b].reshape(1, 32), dtype=np.float32
        )

    key = (C2, mt, B)
    nc = _NC_CACHE.get(key)
    if nc is None:
        nc = _build(C2, mt, num_devices=B)
        _NC_CACHE[key] = nc

    res = run_bass_kernel_spmd(nc, in_maps, core_ids=list(range(B)))
    global LAST_RESULT
    LAST_RESULT = res
    vals = np.array(
        [res.results[b]["loss"][0, 0] for b in range(B)], dtype=np.float32
    )
    return np.float32(np.sum(vals, dtype=np.float32))
